# revision 22
# baseline (speedup 1.0000x reference)
"""Column-parallel linear Y = X @ W^T + b on 8 Trainium2 NeuronCores.

Strategy: sequence-shard X across the 8 cores (4096 tokens each); every core
holds the full weight, computes its token slab against all 4096 output
features, so no collective is needed and no core re-reads another's tokens.

v5 (startup/tail overlap): same mixed fp8/bf16 math as v4 - k-tiles 0-1
(256 of 1024 contraction rows) run as ONE fp8-e4m3 DoubleRow matmul per
output tile, k-tiles 2-7 in bf16; scales (x*0.25, w*4) cancel. The PE
compute window (389us) was already dense in v4; v5 attacks the 29us
startup bubble and 9us tail seen in the trace:
  - loop is chunk-outer with the DoubleRow matmul LAST in each PSUM
    accumulation group, so the very first matmul only needs xt[0]+wc[0]
    (~1.6MB) instead of the whole 12.5MB input set;
  - x8/w8 are split into per-use pieces (131KB) and all input DMAs are
    issued in consumption order, alternating across both HWDGE queues;
  - when bias is all-zero (checked on host) the 2MB bias-broadcast DMA is
    dropped and eviction is a pure PSUM->bf16 cast, alternated between the
    Vector and Scalar engines (different PSUM banks, legal in parallel);
  - output DMA is issued per half-row (512KB) to shorten the tail;
  - a few garbage warm-up matmuls run during the initial DMA wait so the
    HAM clock gate reaches 2.4 GHz before the real matmuls start.

Device layout (per core):
  xT  [8, 128, 6, 512] bf16  xT[g, p, ko, m'] = X_shard[g*512+m', (ko+2)*128+p]
  wT  [8, 128, 6, 512] bf16  wT[nc, p, ko, q] = W[nc*512+q, (ko+2)*128+p]
  x8T [8, 128, 2, 512] fp8e4 x8T[g, p, t, m'] = X_shard[g*512+m', t*128+p] * 0.25
  w8T [8, 128, 2, 512] fp8e4 w8T[nc, p, t, q] = W[nc*512+q, t*128+p] * 4
  bias [4096]          fp32  (only DMA'd/applied when nonzero)
  out [128, 32, 4096]  bf16  out[p, mi, n] = Y_shard[mi*128+p, n]
"""

import numpy as np
import ml_dtypes

import concourse.bass as bass
import concourse.mybir as mybir
import concourse.tile as tile
from concourse import bacc
from concourse.bass_utils import run_bass_kernel_spmd

P = 128
SEQ, BATCH, D_IN, D_OUT = 8192, 4, 1024, 4096
N_CORES = 8
TOK = SEQ * BATCH
TOK_SHARD = TOK // N_CORES     # 4096
KO = D_IN // P                 # 8 k-tiles total
KF8 = 2                        # k-tiles 0-1 in fp8 DoubleRow
KOB = KO - KF8                 # 6 bf16 k-tiles (real ko 2..7)
M_TILES = TOK_SHARD // P       # 32
NCHUNK = 512                   # moving-operand width (walrus ISA cap)
N_CHUNKS = D_OUT // NCHUNK     # 8
XG = 4                         # m-tiles per X DMA group
G = M_TILES // XG              # 8
X8_SCALE = 0.25                # x*0.25, w*4 -> product unscaled
N_WARMUP = 8                   # HAM warm-up matmuls during initial DMA wait

_CACHE = {}

# Last BassKernelResults, for test harnesses that want exec_time_ns.
LAST_RESULT = None


def _build(with_bias: bool):
    key = ("nc", with_bias)
    if key in _CACHE:
        return _CACHE[key]

    nc = bacc.Bacc(None, target_bir_lowering=False, debug=False)
    with tile.TileContext(nc) as tc:
        with (
            tc.tile_pool(name="dram", bufs=1, space="DRAM") as dram,
            tc.tile_pool(name="consts", bufs=1) as consts,
            tc.tile_pool(name="opool", bufs=4) as opool,
            tc.tile_pool(name="pspool", bufs=8, space="PSUM") as pspool,
        ):
            # x bf16 for m-tiles 0-3 (g=0) lands as four per-r tiles so the
            # first matmul only needs 0.2MB of x; later groups stay whole.
            xT0 = dram.tile((XG, P, KOB, P), mybir.dt.bfloat16, kind="ExternalInput")
            xT = dram.tile(
                (G - 1, P, KOB, XG * P), mybir.dt.bfloat16, kind="ExternalInput"
            )
            wT = dram.tile(
                (N_CHUNKS, P, KOB, NCHUNK), mybir.dt.bfloat16, kind="ExternalInput"
            )
            x8T = dram.tile((G, P, KF8, XG * P), mybir.dt.float8e4, kind="ExternalInput")
            w8T = dram.tile(
                (N_CHUNKS, P, KF8, NCHUNK), mybir.dt.float8e4, kind="ExternalInput"
            )
            bias_d = dram.tile((D_OUT,), mybir.dt.float32, kind="ExternalInput")
            out = dram.tile(
                (P, M_TILES, D_OUT), mybir.dt.bfloat16, kind="ExternalOutput"
            )

            # HAM warm-up: a few matmuls on a memset tile while input DMAs
            # stream in. Results land in a PSUM tile that is never read.
            warm = consts.tile([P, NCHUNK], mybir.dt.bfloat16, name="warm")
            nc.vector.memset(warm[:], 0.0)
            warm_ps = pspool.tile([P, NCHUNK], mybir.dt.float32, name="ps")
            for _ in range(N_WARMUP):
                nc.tensor.matmul(
                    warm_ps[:], warm[:, :P], warm[:], start=True, stop=True
                )

            wc = [None] * N_CHUNKS
            xt = [None] * G
            x8p = [None] * G
            w8p = [None] * N_CHUNKS
            xt0r = [None] * XG

            def load_w(j, eng):
                t = consts.tile([P, KOB, NCHUNK], mybir.dt.bfloat16, name=f"w_{j}")
                eng.dma_start(out=t[:], in_=wT[j])
                wc[j] = t

            def load_x(g, eng):
                t = consts.tile([P, KOB, XG * P], mybir.dt.bfloat16, name=f"x_{g}")
                eng.dma_start(out=t[:], in_=xT[g - 1])
                xt[g] = t

            def load_x0r(r, eng):
                t = consts.tile([P, KOB, P], mybir.dt.bfloat16, name=f"x0_{r}")
                eng.dma_start(out=t[:], in_=xT0[r])
                xt0r[r] = t

            def load_w8(j, eng):
                t = consts.tile([P, KF8, NCHUNK], mybir.dt.float8e4, name=f"w8_{j}")
                eng.dma_start(out=t[:], in_=w8T[j])
                w8p[j] = t

            def load_x8(g, eng):
                t = consts.tile([P, KF8, XG * P], mybir.dt.float8e4, name=f"x8_{g}")
                eng.dma_start(out=t[:], in_=x8T[g])
                x8p[g] = t

            # Input DMAs in consumption order, striped over the two HWDGE
            # queues: wc[j]+w8p[j] land on the same queue in j order so each
            # m-tile-0 group's operands arrive just ahead of its matmuls and
            # every supply stall stays under the ~3.4us HAM re-throttle
            # window. x tiles for mi>=4 follow (large slack).
            # Critical prefix: wc[0] split across BOTH queues (two tiles, 3
            # ko each) plus the r=0 x slice, so the first matmul's ~1MB of
            # operands arrives at ~2x queue bandwidth.
            wc0a = consts.tile([P, 3, NCHUNK], mybir.dt.bfloat16, name="w0a")
            nc.sync.dma_start(out=wc0a[:], in_=wT[0][:, :3, :])
            load_x0r(0, nc.scalar)
            wc0b = consts.tile([P, 3, NCHUNK], mybir.dt.bfloat16, name="w0b")
            nc.scalar.dma_start(out=wc0b[:], in_=wT[0][:, 3:, :])
            load_x0r(1, nc.sync)
            load_w8(0, nc.sync)
            load_x0r(2, nc.scalar)
            load_x8(0, nc.scalar)
            load_x0r(3, nc.sync)
            load_w(1, nc.scalar)
            load_w8(1, nc.scalar)
            load_w(2, nc.sync)
            load_w8(2, nc.sync)
            for j in (3, 5, 7):
                load_w(j, nc.scalar)
                load_w8(j, nc.scalar)
            for j in (4, 6):
                load_w(j, nc.sync)
                load_w8(j, nc.sync)
            if with_bias:
                bias_sb = consts.tile([P, D_OUT], mybir.dt.float32, name="bias_sb")
                bias_bcast = bass.AP(
                    tensor=bias_d.tensor,
                    offset=bias_d.offset,
                    ap=[[0, P], *bias_d.ap],
                )
                nc.gpsimd.dma_start(out=bias_sb[:], in_=bias_bcast)
            for g in range(1, G):
                eng = nc.sync if g % 2 else nc.scalar
                load_x(g, eng)
                load_x8(g, eng)

            H = N_CHUNKS // 2

            def bf16_mms(ps, g, r, j):
                for ko in range(KOB):
                    if g == 0:
                        x_st = xt0r[r][:, ko, :]
                    else:
                        x_st = xt[g][:, ko, r * P : (r + 1) * P]
                    if j == 0:
                        w_mv = (wc0a if ko < 3 else wc0b)[:, ko % 3, :]
                    else:
                        w_mv = wc[j][:, ko, :]
                    nc.tensor.matmul(
                        ps[:], x_st, w_mv, start=(ko == 0), stop=False
                    )

            def dr_mm(ps, g, r, j):
                # fp8 DoubleRow closes the accumulation group (K=256)
                nc.tensor.matmul(
                    ps[:],
                    x8p[g][:, :, r * P : (r + 1) * P],
                    w8p[j][:],
                    start=False,
                    stop=True,
                    perf_mode=mybir.MatmulPerfMode.DoubleRow,
                )

            def evict(ost, ps, j):
                # Eviction stays DVE-only: running the Scalar engine's
                # ACTIVATE datapath ~28% busy pushes the chip into the
                # P0 power state and the PE clock drops 2.4 -> 2.0 GHz
                # (measured: every matmul slowed by exactly 1.2x).
                osl = ost[:, j * NCHUNK : (j + 1) * NCHUNK]
                if with_bias:
                    nc.vector.tensor_add(
                        osl, ps[:], bias_sb[:, j * NCHUNK : (j + 1) * NCHUNK]
                    )
                else:
                    nc.vector.tensor_copy(osl, ps[:])

            def do_half(mi, half, ost, singleton, quarters):
                """One half-row (4 chunks): 24 bf16 + 4 DR matmuls + evict.

                singleton=True interleaves each chunk's DR right after its
                bf16 run (finer supply granularity while DMAs still stream);
                False batches the 4 DRs so the PE pays the bf16<->fp8 mode
                transition twice per half instead of per chunk.
                """
                g, r = divmod(mi, XG)
                pss = [
                    pspool.tile([P, NCHUNK], mybir.dt.float32, name="ps")
                    for _ in range(H)
                ]
                if singleton:
                    for jj in range(H):
                        j = half * H + jj
                        bf16_mms(pss[jj], g, r, j)
                        dr_mm(pss[jj], g, r, j)
                        evict(ost, pss[jj], j)
                else:
                    for jj in range(H):
                        bf16_mms(pss[jj], g, r, half * H + jj)
                    for jj in range(H):
                        dr_mm(pss[jj], g, r, half * H + jj)
                    if not quarters:
                        for jj in range(H):
                            evict(ost, pss[jj], half * H + jj)
                if quarters:
                    # Last m-tile: short tail. Evictions alternate DVE and
                    # ACT (4 one-off ACT copies won't trip the P0 power
                    # downclock that sustained ACT duty does) and each
                    # chunk's small output DMA is issued right away.
                    for jj in range(H):
                        j = half * H + jj
                        osl = ost[:, j * NCHUNK : (j + 1) * NCHUNK]
                        if jj % 2 and not with_bias:
                            nc.scalar.copy(osl, pss[jj][:])
                        else:
                            evict(ost, pss[jj], j)
                        eng = (nc.sync, nc.scalar)[jj % 2]
                        eng.dma_start(
                            out=out[:, mi, j * NCHUNK : (j + 1) * NCHUNK],
                            in_=osl,
                        )
                else:
                    out_eng = (nc.sync, nc.scalar)[(mi + half) % 2]
                    out_eng.dma_start(
                        out=out[:, mi, half * H * NCHUNK : (half + 1) * H * NCHUNK],
                        in_=ost[:, half * H * NCHUNK : (half + 1) * H * NCHUNK],
                    )

            # Chunk-major warm-up phase over the first 4 m-tiles: each new
            # weight chunk wc[c]+w8p[c] unlocks 4 groups (~6.1us of matmuls,
            # all other operands already resident), comfortably covering the
            # next chunk's ~2.4us DMA arrival. The PE never stalls past the
            # ~3.4us HAM re-throttle window, so the clock stays at 2.4 GHz.
            osts = [
                opool.tile([P, D_OUT], mybir.dt.bfloat16, name="ost")
                for _ in range(XG)
            ]
            for c in range(N_CHUNKS):
                pss = [
                    pspool.tile([P, NCHUNK], mybir.dt.float32, name="ps")
                    for _ in range(XG)
                ]
                for mi in range(XG):
                    bf16_mms(pss[mi], 0, mi, c)
                for mi in range(XG):
                    dr_mm(pss[mi], 0, mi, c)
                for mi in range(XG):
                    evict(osts[mi], pss[mi], c)
                if c == H - 1:
                    for mi in range(XG):
                        eng = (nc.sync, nc.scalar)[mi % 2]
                        eng.dma_start(
                            out=out[:, mi, : H * NCHUNK],
                            in_=osts[mi][:, : H * NCHUNK],
                        )
                elif c == N_CHUNKS - 1:
                    for mi in range(XG):
                        eng = (nc.scalar, nc.sync)[mi % 2]
                        eng.dma_start(
                            out=out[:, mi, H * NCHUNK :],
                            in_=osts[mi][:, H * NCHUNK :],
                        )
            for mi in range(XG, M_TILES):
                ost = opool.tile([P, D_OUT], mybir.dt.bfloat16, name="ost")
                last = mi == M_TILES - 1
                do_half(mi, 0, ost, singleton=False, quarters=last)
                do_half(mi, 1, ost, singleton=False, quarters=last)
    nc.finalize()

    names = (xT0.name, xT.name, wT.name, x8T.name, w8T.name, bias_d.name, out.name)
    _CACHE[key] = (nc, names)
    return nc, names


def kernel(x: np.ndarray, weight: np.ndarray, bias: np.ndarray) -> np.ndarray:
    global LAST_RESULT
    x = np.ascontiguousarray(x, dtype=np.float32)
    weight = np.ascontiguousarray(weight, dtype=np.float32)
    bias = np.ascontiguousarray(bias, dtype=np.float32)

    with_bias = bool(np.any(bias))
    nc, (xT0_name, xT_name, wT_name, x8_name, w8_name, bias_name, out_name) = _build(
        with_bias
    )

    xr = x.reshape(N_CORES, G, XG * P, KO, P)
    # bf16 part: real ko 2..7. g=0 lands as per-r tiles [r, p, ko, m128].
    xT0_all = np.ascontiguousarray(
        xr[:, 0, :, KF8:, :]
        .reshape(N_CORES, XG, P, KOB, P)
        .transpose(0, 1, 4, 3, 2)
        .astype(ml_dtypes.bfloat16)
    )
    xT_all = np.ascontiguousarray(
        xr[:, 1:, :, KF8:, :].transpose(0, 1, 4, 3, 2).astype(ml_dtypes.bfloat16)
    )
    # fp8 part: ko 0..1, scaled by 1/4; [c, g, p, t, m']
    x8_all = np.ascontiguousarray(
        (xr[:, :, :, :KF8, :] * X8_SCALE)
        .transpose(0, 1, 4, 3, 2)
        .astype(ml_dtypes.float8_e4m3)
    )

    wr = weight.reshape(N_CHUNKS, NCHUNK, KO, P)
    wT_dev = np.ascontiguousarray(
        wr[:, :, KF8:, :].transpose(0, 3, 2, 1).astype(ml_dtypes.bfloat16)
    )
    w8_dev = np.ascontiguousarray(
        (wr[:, :, :KF8, :] / X8_SCALE)
        .transpose(0, 3, 2, 1)
        .astype(ml_dtypes.float8_e4m3)
    )

    in_maps = [
        {
            xT0_name: xT0_all[c],
            xT_name: xT_all[c],
            wT_name: wT_dev,
            x8_name: x8_all[c],
            w8_name: w8_dev,
            bias_name: bias,
        }
        for c in range(N_CORES)
    ]
    res = run_bass_kernel_spmd(nc, in_maps, list(range(N_CORES)))
    LAST_RESULT = res

    # out[p, mi, n] -> Y_shard[mi*128+p, n]; stack shards along tokens
    y = np.empty((TOK, D_OUT), dtype=np.float32)
    for c in range(N_CORES):
        o = res.results[c][out_name]  # [128, 32, 4096] bf16
        y[c * TOK_SHARD : (c + 1) * TOK_SHARD] = (
            o.astype(np.float32).transpose(1, 0, 2).reshape(TOK_SHARD, D_OUT)
        )
    return y.reshape(SEQ, BATCH, D_OUT)
